# revision 1
# baseline (speedup 1.0000x reference)
"""Trainium2 Bass kernel for the NeighborhoodAttention module.

Data-parallel over B across 8 cores; all_embs + weights replicated.

Math (per batch row b):
    center = E[ci[b]]; nbs[j] = E[ni[b,j]]
    qk     = (center @ Wq.T) @ Wk          (avoids projecting all K neighbors)
    logits[j] = SCALE * <nbs[j], qk> + log(clip(w[b,j]))
    attn   = softmax(logits)               (no max-sub needed: logits <= ~3)
    gate   = sigmoid(center @ Wg.T + bg)
    ctx    = gate * sum_j attn[j]*nbs[j]
    x      = center @ (Wo1+I).T + ctx @ Wo2.T + bo   (residual folded into Wo1)
    out    = (x-mu)/sqrt(var+eps)          (gamma/beta applied on host)

Device layout per 128-row tile (j-major gather):
    nbs tile [128 b, 16 j, 256 d]  (gather position i = j*128 + b)
    logits[b,j] via DVE tensor_tensor_reduce(nbs[:,j,:], qk) per j.
    context on PE: 16 accumulating matmuls, stationary_j = diag(attn[:,j]),
    built in one shot as attn-broadcast * identity-broadcast on GPSIMD.
"""

import os
from contextlib import ExitStack

import numpy as np

import concourse.bass as bass
import concourse.tile as tile
from concourse import bacc, mybir

D = 256
A = 64
K = 16
N = 20000
B = 32768
NCORES = 8
BC = B // NCORES          # 4096 rows per core
SCALE = A ** -0.5
EPS = 1e-5

f32 = mybir.dt.float32
f32r = mybir.dt.float32r
i16 = mybir.dt.int16

FN = mybir.ActivationFunctionType
OP = mybir.AluOpType
AX = mybir.AxisListType


def r(ap):
    """bitcast an AP to float32r (fast fp32 matmul mode)."""
    return ap.bitcast(f32r)


def build_program(tiles=BC // 128, stage=7):
    """Builds the per-core program.  `tiles` = number of 128-row tiles."""
    nc = bacc.Bacc("TRN2", target_bir_lowering=False, debug=False)
    pairs = tiles // 2
    assert tiles % 2 == 0

    embs = nc.dram_tensor("embs", [N, D], f32r, kind="ExternalInput")
    nbs_idx = nc.dram_tensor("nbs_idx", [128, tiles * 128], i16, kind="ExternalInput")
    ctr_idx = nc.dram_tensor("ctr_idx", [128, pairs * 16], i16, kind="ExternalInput")
    nbw = nc.dram_tensor("nbw", [128, tiles * 16], f32, kind="ExternalInput")
    wqT = nc.dram_tensor("wqT", [128, 2, A], f32r, kind="ExternalInput")
    wk = nc.dram_tensor("wk", [A, D], f32r, kind="ExternalInput")
    wgT = nc.dram_tensor("wgT", [128, 2, D], f32r, kind="ExternalInput")
    w1T = nc.dram_tensor("w1T", [128, 2, D], f32r, kind="ExternalInput")
    w2T = nc.dram_tensor("w2T", [128, 2, D], f32r, kind="ExternalInput")
    bg_bo = nc.dram_tensor("bg_bo", [1, 2 * D], f32r, kind="ExternalInput")
    ident = nc.dram_tensor("ident", [128, 128], f32, kind="ExternalInput")
    ones1 = nc.dram_tensor("ones1", [1, 128], f32r, kind="ExternalInput")
    out_d = nc.dram_tensor("out", [tiles * 128, D], f32, kind="ExternalOutput")

    with tile.TileContext(nc) as tc, ExitStack() as ctx:
        const = ctx.enter_context(tc.tile_pool(name="const", bufs=1))
        idx_sb = const.tile([128, tiles * 128], i16)
        cidx_sb = const.tile([128, pairs * 16], i16)
        nbw_sb = const.tile([128, tiles * 16], f32)
        wqT_sb = const.tile([128, 2, A], f32r)
        wk_sb = const.tile([A, D], f32r)
        wgT_sb = const.tile([128, 2, D], f32r)
        w1T_sb = const.tile([128, 2, D], f32r)
        w2T_sb = const.tile([128, 2, D], f32r)
        bgbo_sb = const.tile([1, 2 * D], f32r)
        id_sb = const.tile([128, 128], f32)
        ones1_sb = const.tile([1, 128], f32r)
        eps_sb = const.tile([128, 1], f32)
        nc.vector.memset(eps_sb[:], EPS)
        for sb, dr in [(idx_sb, nbs_idx), (cidx_sb, ctr_idx), (nbw_sb, nbw),
                       (wqT_sb, wqT), (wk_sb, wk), (wgT_sb, wgT), (w1T_sb, w1T),
                       (w2T_sb, w2T), (bgbo_sb, bg_bo), (id_sb, ident),
                       (ones1_sb, ones1)]:
            nc.sync.dma_start(sb[:], dr.ap())
        bg_row = bgbo_sb[:, 0:D]
        bo_row = bgbo_sb[:, D:2 * D]

        nbs_p = ctx.enter_context(tc.tile_pool(name="nbs", bufs=3))
        ctr_p = ctx.enter_context(tc.tile_pool(name="ctr", bufs=2))
        sb_p = ctx.enter_context(tc.tile_pool(name="work", bufs=2))
        small_p = ctx.enter_context(tc.tile_pool(name="small", bufs=3))
        ps_p = ctx.enter_context(tc.tile_pool(name="ps", bufs=1, space="PSUM"))

        for pr in range(pairs):
            # ---- center gather + transpose (pair granularity) --------------
            ctr_t = ctr_p.tile([128, 2, D], f32r, tag="ctr")
            nc.gpsimd.dma_gather(
                ctr_t[:], embs.ap(), cidx_sb[:, pr * 16:(pr + 1) * 16],
                256, 256, D)
            # centerT for the pair: [128 d_lo, 2 d_hi, 2 tile, 128 b]
            cT2 = sb_p.tile([128, 2, 2, 128], f32r, tag="cT2")
            for i in range(2):
                cT_ps = ps_p.tile([128, 256], f32, tag="cT_ps", name="cT_ps")
                for c in range(2):
                    nc.tensor.transpose(cT_ps[:, c * 128:(c + 1) * 128],
                                        ctr_t[:, i, c * 128:(c + 1) * 128].bitcast(f32),
                                        id_sb[:])
                for c in range(2):
                    nc.scalar.copy(cT2[:, c, i, :],
                                   cT_ps[:, c * 128:(c + 1) * 128])
            # q for the pair: qT [a, (tile,b)]
            qT_ps = ps_p.tile([64, 256], f32, tag="qT_ps", name="qT_ps")
            nc.tensor.matmul(qT_ps[:], wqT_sb[:, 0, :],
                             cT2[:, 0, :, :], start=True, stop=False)
            nc.tensor.matmul(qT_ps[:], wqT_sb[:, 1, :],
                             cT2[:, 1, :, :], start=False, stop=True)
            qT_sb = small_p.tile([64, 2, 128], f32r, tag="qT")
            nc.scalar.copy(qT_sb[:], qT_ps[:])

            for i in range(2):
                t = 2 * pr + i
                nbs_t = nbs_p.tile([128, K, D], f32r, tag="nbs")
                nc.gpsimd.dma_gather(
                    nbs_t[:], embs.ap(), idx_sb[:, t * 128:(t + 1) * 128],
                    2048, 2048, D, single_packet=False)

                if stage < 2:
                    xn = sb_p.tile([128, D], f32, tag="xn")
                    nc.vector.tensor_scalar_mul(xn[:], nbs_t[:, 0, :].bitcast(f32), 1.0)
                    nc.sync.dma_start(out_d.ap()[t * 128:(t + 1) * 128, :], xn[:])
                    continue
                # qk rows for this tile: [128 b, 256 d]
                qk_ps = ps_p.tile([128, 256], f32, tag="qk_ps", name="qk_ps")
                nc.tensor.matmul(qk_ps[:], qT_sb[:, i, :], wk_sb[:],
                                 start=True, stop=True)
                qk_sb = sb_p.tile([128, D], f32, tag="qk")
                nc.scalar.copy(qk_sb[:], qk_ps[:])

                if stage < 3:
                    xn = sb_p.tile([128, D], f32, tag="xn")
                    nc.vector.tensor_scalar_mul(xn[:], qk_sb[:], 1.0)
                    nc.sync.dma_start(out_d.ap()[t * 128:(t + 1) * 128, :], xn[:])
                    continue
                # ---- logits ------------------------------------------------
                logits = small_p.tile([128, K], f32, tag="logits")
                scratch = sb_p.tile([128, D], f32, tag="scratch")
                for j in range(K):
                    nc.vector.scalar_tensor_tensor(
                        out=scratch[:], in0=nbs_t[:, j, :].bitcast(f32),
                        scalar=SCALE, in1=qk_sb[:],
                        op0=OP.mult, op1=OP.mult,
                        accum_out=logits[:, j:j + 1])

                if stage < 4:
                    xn = sb_p.tile([128, D], f32, tag="xn")
                    nc.vector.tensor_scalar_mul(xn[:], scratch[:], 1.0)
                    nc.sync.dma_start(out_d.ap()[t * 128:(t + 1) * 128, :], xn[:])
                    continue
                # ---- softmax (w/ neighbor-weight log bias) -----------------
                clipw = small_p.tile([128, K], f32, tag="clipw")
                nc.vector.tensor_scalar_max(clipw[:],
                                            nbw_sb[:, t * 16:(t + 1) * 16],
                                            1e-6)
                logw = small_p.tile([128, K], f32, tag="logw")
                nc.scalar.activation(logw[:], clipw[:], FN.Ln)
                biased = small_p.tile([128, K], f32, tag="biased")
                nc.vector.tensor_tensor(biased[:], logits[:], logw[:],
                                        op=OP.add)
                exps = small_p.tile([128, K], f32, tag="exps")
                sums = small_p.tile([128, 1], f32, tag="sums")
                nc.scalar.activation(exps[:], biased[:], FN.Exp,
                                     accum_out=sums[:])
                recip = small_p.tile([128, 1], f32, tag="recip")
                nc.vector.reciprocal(recip[:], sums[:])
                attn = small_p.tile([128, K], f32, tag="attn")
                nc.vector.tensor_scalar(attn[:], exps[:], recip[:], None,
                                        op0=OP.mult)
                if stage < 5:
                    xn = sb_p.tile([128, D], f32, tag="xn")
                    nc.vector.tensor_scalar_mul(xn[:, 0:K], attn[:], 1.0)
                    nc.sync.dma_start(out_d.ap()[t * 128:(t + 1) * 128, :], xn[:])
                    continue
                # 16 diagonal stationaries in one shot: [128, 16 j, 128 m]
                diag_all = sb_p.tile([128, K, 128], f32r, tag="diag_all")
                nc.vector.tensor_tensor(
                    diag_all[:],
                    attn[:].unsqueeze(2).broadcast_to([128, K, 128]),
                    id_sb[:].unsqueeze(1).broadcast_to([128, K, 128]),
                    op=OP.mult)

                # ---- context: 16 accumulating diag matmuls -----------------
                ctx_ps = ps_p.tile([128, 256], f32, tag="ctx_ps", name="ctx_ps", bufs=2)
                for j in range(K):
                    nc.tensor.matmul(ctx_ps[:], diag_all[:, j, :],
                                     nbs_t[:, j, :],
                                     start=(j == 0), stop=(j == K - 1))

                if stage < 6:
                    xn = sb_p.tile([128, D], f32, tag="xn")
                    nc.vector.tensor_scalar_mul(xn[:], ctx_ps[:], 1.0)
                    nc.sync.dma_start(out_d.ap()[t * 128:(t + 1) * 128, :], xn[:])
                    continue
                # ---- gate --------------------------------------------------
                gate_ps = ps_p.tile([128, 256], f32, tag="gate_ps", name="gate_ps")
                nc.tensor.matmul(gate_ps[:], cT2[:, 0, i, :],
                                 wgT_sb[:, 0, :], start=True, stop=False)
                nc.tensor.matmul(gate_ps[:], cT2[:, 1, i, :],
                                 wgT_sb[:, 1, :], start=False, stop=False)
                nc.tensor.matmul(gate_ps[:], ones1_sb[:], bg_row,
                                 start=False, stop=True)
                gate_sb = sb_p.tile([128, D], f32, tag="gate")
                nc.scalar.activation(gate_sb[:], gate_ps[:], FN.Sigmoid)

                ctxg = sb_p.tile([128, D], f32, tag="ctxg")
                nc.vector.tensor_tensor(ctxg[:], gate_sb[:], ctx_ps[:],
                                        op=OP.mult)
                # transpose gated context for the output projection
                cgT_ps = ps_p.tile([128, 256], f32, tag="cgT_ps", name="cgT_ps")
                nc.tensor.transpose(cgT_ps[:, 0:128], ctxg[:, 0:128], id_sb[:])
                nc.tensor.transpose(cgT_ps[:, 128:256], ctxg[:, 128:256],
                                    id_sb[:])
                cgT = sb_p.tile([128, 2, 128], f32r, tag="cgT")
                nc.scalar.copy(cgT[:, 0, :], cgT_ps[:, 0:128])
                nc.scalar.copy(cgT[:, 1, :], cgT_ps[:, 128:256])

                if stage < 7:
                    xn = sb_p.tile([128, D], f32, tag="xn")
                    nc.vector.tensor_scalar_mul(xn[:], ctxg[:], 1.0)
                    nc.sync.dma_start(out_d.ap()[t * 128:(t + 1) * 128, :], xn[:])
                    continue
                # ---- output projection + residual (folded) + bias ----------
                x_ps = ps_p.tile([128, 256], f32, tag="x_ps", name="x_ps")
                nc.tensor.matmul(x_ps[:], cT2[:, 0, i, :], w1T_sb[:, 0, :],
                                 start=True, stop=False)
                nc.tensor.matmul(x_ps[:], cT2[:, 1, i, :], w1T_sb[:, 1, :],
                                 start=False, stop=False)
                nc.tensor.matmul(x_ps[:], cgT[:, 0, :], w2T_sb[:, 0, :],
                                 start=False, stop=False)
                nc.tensor.matmul(x_ps[:], cgT[:, 1, :], w2T_sb[:, 1, :],
                                 start=False, stop=False)
                nc.tensor.matmul(x_ps[:], ones1_sb[:], bo_row,
                                 start=False, stop=True)

                # ---- layernorm --------------------------------------------
                bnst = small_p.tile([128, 6], f32, tag="bnst")
                nc.vector.bn_stats(bnst[:], x_ps[:])
                bnag = small_p.tile([128, 2], f32, tag="bnag")
                nc.vector.bn_aggr(bnag[:], bnst[:])
                sd = small_p.tile([128, 1], f32, tag="sd")
                nc.scalar.activation(sd[:], bnag[:, 1:2], FN.Sqrt,
                                     bias=eps_sb[:])
                rs = small_p.tile([128, 1], f32, tag="rs")
                nc.vector.reciprocal(rs[:], sd[:])
                xn = sb_p.tile([128, D], f32, tag="xn")
                nc.vector.tensor_scalar(xn[:], x_ps[:], bnag[:, 0:1], rs[:],
                                        op0=OP.subtract, op1=OP.mult)
                nc.sync.dma_start(out_d.ap()[t * 128:(t + 1) * 128, :], xn[:])

    nc.compile()
    return nc


# ---------------------------------------------------------------------------
# host-side input marshalling
# ---------------------------------------------------------------------------

def prep_core_inputs(all_embs, center_idx, nb_idx, nb_weights,
                     Wq, Wk, Wg, bg, Wo, bo, tiles=BC // 128):
    """Returns (shared_inputs, per_core_list) of numpy arrays."""
    bc = tiles * 128
    pairs = tiles // 2
    ncores = B // BC if bc == BC else 1

    WqT = np.ascontiguousarray(Wq.T.astype(np.float32))              # [D, A]
    WgT = np.ascontiguousarray(Wg.T.astype(np.float32))              # [D, D]
    W1 = Wo[:, :D].astype(np.float32) + np.eye(D, dtype=np.float32)
    W1T = np.ascontiguousarray(W1.T)
    W2T = np.ascontiguousarray(Wo[:, D:].astype(np.float32).T)

    def chunk2(m):  # [D, X] -> [128, 2, X]
        return np.ascontiguousarray(m.reshape(2, 128, -1).transpose(1, 0, 2))

    shared = dict(
        embs=np.ascontiguousarray(all_embs.astype(np.float32)),
        wqT=chunk2(WqT), wk=np.ascontiguousarray(Wk.astype(np.float32)),
        wgT=chunk2(WgT), w1T=chunk2(W1T), w2T=chunk2(W2T),
        bg_bo=np.concatenate([bg, bo]).astype(np.float32)[None, :],
        ident=np.eye(128, dtype=np.float32),
        ones1=np.ones((1, 128), np.float32),
    )

    def wrap16(flat):
        """flat [n_idx] in gather-position order -> [16, n/16] (pos i at
        [i % 16, i // 16]); caller concatenates tiles and tiles x8."""
        return flat.reshape(-1, 16).T

    per_core = []
    for c in range(ncores):
        rows = slice(c * bc, (c + 1) * bc)
        nb = nb_idx[rows].astype(np.int64).reshape(tiles, 128, K)
        # j-major gather order per tile: position i = j*128 + b
        nmat = np.concatenate(
            [wrap16(nb[t].T.reshape(-1)) for t in range(tiles)], axis=1)
        nmat = np.ascontiguousarray(np.tile(nmat, (8, 1)).astype(np.int16))

        ct = center_idx[rows].astype(np.int64)        # [bc]
        cmat = np.concatenate(
            [wrap16(ct[p * 256:(p + 1) * 256]) for p in range(pairs)], axis=1)
        cmat = np.ascontiguousarray(np.tile(cmat, (8, 1)).astype(np.int16))

        w = nb_weights[rows].astype(np.float32).reshape(tiles, 128, K)
        wf = np.ascontiguousarray(
            w.transpose(1, 0, 2).reshape(128, tiles * K))

        per_core.append(dict(nbs_idx=nmat, ctr_idx=cmat, nbw=wf))
    return shared, per_core


_CACHE = {}


def kernel(all_embs, center_idx, nb_idx, nb_weights, Wq, Wk, Wg, bg, Wo, bo,
           gamma, beta):
    from concourse.bass_utils import run_bass_kernel_spmd

    key = "full"
    if key not in _CACHE:
        _CACHE[key] = build_program()
    nc = _CACHE[key]

    shared, per_core = prep_core_inputs(
        np.asarray(all_embs), np.asarray(center_idx), np.asarray(nb_idx),
        np.asarray(nb_weights), np.asarray(Wq), np.asarray(Wk),
        np.asarray(Wg), np.asarray(bg), np.asarray(Wo), np.asarray(bo))

    in_maps = [{**shared, **pc} for pc in per_core]
    res = run_bass_kernel_spmd(nc, in_maps, list(range(NCORES)),
                               trace=bool(int(os.environ.get("KTRACE", "0"))))
    out = np.concatenate([res.results[c]["out"] for c in range(NCORES)],
                         axis=0)
    g = np.asarray(gamma, np.float32)
    bt = np.asarray(beta, np.float32)
    if not (np.all(g == 1.0) and np.all(bt == 0.0)):
        out = out * g[None, :] + bt[None, :]
    kernel.last_results = res
    return out.astype(np.float32)

